# revision 15
# baseline (speedup 1.0000x reference)
"""Multi-head attention (B=4, S=2048, H=1024, 16 heads) on 8 trn2 NeuronCores.

Sharding: data-parallel over batch (4) x tensor-parallel over head-groups (2):
core c handles batch c//2, heads 8*(c%2) .. 8*(c%2)+8. Each core computes its
partial output projection; host sums the two head-group partials + bo.

Per-core device algorithm (all matmul inputs bf16, fp32 accumulation):
  inputs are pre-transposed on host: xqT/xkT/xvT = x^T (hidden, seq)
  QT[t] (128=2 heads' d, sq)   = wqT-chunks^T @ xqT-chunks (+bias, per-partition)
  KT[t] likewise
  V[i]  (128 sk, 8*(64+1))     = xvT-chunks^T @ wvT (+bias), with a ones column
                                 appended per head (for softmax row sums)
  per head-pair t, sq-chunk s (512), sk-tile i (128):
     S^T = KT-slice^T @ QT-slice  (two heads row-packed in the PE array)
     P^T = exp(S^T * 0.125)       (ACT, psum->sbuf, bf16 out)
     ctx (128, 512) += V-block^T @ P^T: V's per-head block is [values|ones],
        so psum rows 0:64 = ctx and rows 64:128 = the softmax denominator
        replicated -- broadcast for free (matmul time depends only on N)
  normalize: ctx psum -> sbuf copy, reciprocal of rows 64:128 (DVE),
     CX = ctx * recip (bf16)
  out (sq, 1024) = sum_t CX-chunks^T @ woT   -> DMA out (fp32)

The attention inner loop is ACT-bound (33.5M exps/core at 1 elem/lane/cycle
@1.2GHz is a ~285us floor); projections for head-pair t+1 and the output
projection are emitted so the Tile scheduler overlaps them into the
ACT-bound gaps. PSUM budget (8 banks): 2 proj + 4 S^T staging + 2 ctx.
"""
import os
import sys

sys.path.insert(0, "/opt/trn_rl_repo")

import numpy as np
import ml_dtypes

import concourse.bass as bass
import concourse.mybir as mybir
import concourse.tile as tile

# ---------------------------------------------------------------------------
# Walrus in this environment allows at most 1 sync wait per instruction (2 for
# EventSemaphore); Tile sometimes emits more (e.g. the exit drain). Hoist the
# extra waits onto EventSemaphore instructions inserted before the offender.
import json as _json


def _transform_bir_json(bir_bytes: bytes) -> bytes:
    bir = _json.loads(bir_bytes)
    changed = False
    ctr = 0
    for fn in bir.get("functions", []):
        for blk in fn.get("blocks", []):
            out = []
            for inst in blk.get("instructions", []):
                si = inst.get("sync_info") or {}
                waits = si.get("on_wait") or []
                cap = 2 if inst.get("opcode") == "EventSemaphore" else 1
                if len(waits) > cap:
                    changed = True
                    extra = waits[:-cap]
                    si["on_wait"] = waits[-cap:]
                    for i in range(0, len(extra), 2):
                        ctr += 1
                        out.append(
                            {
                                "debug": inst.get("debug"),
                                "engine": inst["engine"],
                                "ins": [],
                                "name": f"{inst['name']}_xw{ctr}",
                                "opcode": "EventSemaphore",
                                "outs": [],
                                "sync_info": {
                                    "on_update": [],
                                    "on_wait": extra[i : i + 2],
                                },
                            }
                        )
                out.append(inst)
            blk["instructions"] = out
    if not changed:
        return bir_bytes
    return _json.dumps(bir).encode()


def _apply_bir_patch():
    import concourse.bass_utils as bu
    import concourse.bass2jax as b2j

    if getattr(b2j, "_bir_waitfix_applied", False):
        return
    orig = bu.compile_bir_kernel

    def patched(bir_json, tmpdir, neff_name="file.neff"):
        return orig(_transform_bir_json(bir_json), tmpdir, neff_name)

    b2j.compile_bir_kernel = patched
    bu.compile_bir_kernel = patched
    b2j._bir_waitfix_applied = True


_apply_bir_patch()

from concourse.bass_utils import run_bass_kernel_spmd  # noqa: E402

# ---------------------------------------------------------------------------
HIDDEN = 1024
HEADS = 16
HD = 64  # head dim
B, SQ, SK = 4, 2048, 2048
NCORES = 8
HPC = 8  # heads per core (tensor-parallel over 2 head groups)
HL = HPC * HD  # local hidden slice = 512
SCALE = HD ** -0.5

F32 = mybir.dt.float32
BF16 = mybir.dt.bfloat16

_CACHED = {}


def _build_nc(dt_mm):
    nc = bass.Bass()
    xqT_d = nc.declare_dram_parameter("xqT", [HIDDEN, SQ], dt_mm, isOutput=False)
    xkT_d = nc.declare_dram_parameter("xkT", [HIDDEN, SK], dt_mm, isOutput=False)
    xvT_d = nc.declare_dram_parameter("xvT", [HIDDEN, SK], dt_mm, isOutput=False)
    wqT_d = nc.declare_dram_parameter("wqT", [HIDDEN, HL], dt_mm, isOutput=False)
    wkT_d = nc.declare_dram_parameter("wkT", [HIDDEN, HL], dt_mm, isOutput=False)
    wvT_d = nc.declare_dram_parameter("wvT", [HIDDEN, HL], dt_mm, isOutput=False)
    woT_d = nc.declare_dram_parameter("woT", [HL, HIDDEN], dt_mm, isOutput=False)
    bq_d = nc.declare_dram_parameter("bq2", [128, 4], F32, isOutput=False)
    bk_d = nc.declare_dram_parameter("bk2", [128, 4], F32, isOutput=False)
    bvb_d = nc.declare_dram_parameter("bvb", [128, HL], F32, isOutput=False)
    out_d = nc.declare_dram_parameter("out", [SQ, HIDDEN], F32, isOutput=True)

    NHC = HIDDEN // 128  # 8 hidden chunks
    NT = 4  # head-pair tiles (8 local heads -> 4 pairs of 64 rows)
    NS = 4  # sq chunks of 512
    NI = SK // 128  # 16 sk tiles

    with tile.TileContext(nc) as tc:
        from contextlib import ExitStack

        with ExitStack() as stack:
            wpool = stack.enter_context(tc.tile_pool(name="wpool", bufs=1))
            apool = stack.enter_context(tc.tile_pool(name="apool", bufs=1))

            # ---- persistent weights / biases (DMAs emitted at point of need)
            # wq/wk are split per (hidden-chunk, head-pair) so phase t only
            # waits on its own 32KB slices (t=0's slices gate the first
            # score matmuls of the whole kernel).
            wq_sb = [
                [wpool.tile([128, 128], dt_mm, name=f"wq{c}_{t}", tag=f"wq{c}_{t}") for t in range(NT)]
                for c in range(NHC)
            ]
            wk_sb = [
                [wpool.tile([128, 128], dt_mm, name=f"wk{c}_{t}", tag=f"wk{c}_{t}") for t in range(NT)]
                for c in range(NHC)
            ]
            wv_sb = [wpool.tile([128, HL], dt_mm, name=f"wv{c}", tag=f"wv{c}") for c in range(NHC)]
            wo_sb = [wpool.tile([128, HIDDEN], dt_mm, name=f"wo{t}", tag=f"wo{t}") for t in range(NT)]
            bq_sb = wpool.tile([128, 4], F32)
            bk_sb = wpool.tile([128, 4], F32)
            bvb_sb = wpool.tile([128, HL], F32)

            def emit_w_dma(w4, wT_d, t):
                for c in range(NHC):
                    nc.sync.dma_start(
                        out=w4[c][t][:],
                        in_=wT_d[128 * c : 128 * c + 128, 128 * t : 128 * t + 128],
                    )

            # ---- persistent activations
            QT = [apool.tile([128, SQ], dt_mm, name=f"QT{t}", tag=f"QT{t}") for t in range(NT)]
            # t=0's Q tiles are split per s-chunk so QT(0,s) projections can
            # be scheduled during attention(0,s') without a same-tile
            # write-during-read hazard.
            QT0s = [apool.tile([128, 512], dt_mm, name=f"QT0s{s}", tag=f"QT0s{s}") for s in range(NS)]
            # KT split per sk-chunk so attention(t,s,i) only depends on the
            # K projection covering its own sk range (lets the first
            # attention chunks start before all of K(t) is projected).
            KT = [
                [
                    apool.tile([128, 512], dt_mm, name=f"KT{t}_{c}", tag=f"KT{t}_{c}")
                    for c in range(NS)
                ]
                for t in range(NT)
            ]
            # V[i]: per-head 128-col block [0:64]=V values, [64:128]=ones.
            # The ones half makes the ctx matmul emit the softmax denominator
            # replicated across psum rows 64..127 at zero extra PE cost
            # (matmul time depends only on N).
            V = [apool.tile([128, HPC * 128], dt_mm, name=f"V{i}", tag=f"V{i}") for i in range(NI)]
            CX = [apool.tile([128, SQ], dt_mm, name=f"CX{t}", tag=f"CX{t}") for t in range(NT)]

            inner = stack.enter_context(ExitStack())
            spool = inner.enter_context(tc.tile_pool(name="ldpool", bufs=2))
            dpool = inner.enter_context(tc.tile_pool(name="dpool", bufs=4))
            psA = inner.enter_context(tc.tile_pool(name="psA", bufs=2, space="PSUM"))
            psS = inner.enter_context(tc.tile_pool(name="psS", bufs=2, space="PSUM"))
            psC = inner.enter_context(tc.tile_pool(name="psC", bufs=1, space="PSUM"))

            def emit_proj(t, s, which):
                xT_d, w_sb, b_sb, OUT, nm = which
                xch = [
                    spool.tile([128, 512], dt_mm, name=f"x{nm}{t}{s}_{c}", tag=f"xch{c}")
                    for c in range(NHC)
                ]
                for c in range(NHC):
                    nc.sync.dma_start(
                        out=xch[c][:],
                        in_=xT_d[128 * c : 128 * c + 128, 512 * s : 512 * s + 512],
                    )
                ps = psA.tile([128, 512], F32, name=f"ps{nm}{s}{t}", tag="psA")
                for c in range(NHC):
                    nc.tensor.matmul(
                        ps[:],
                        w_sb[c][t][:],
                        xch[c][:],
                        start=(c == 0),
                        stop=(c == NHC - 1),
                    )
                if t == 0 and OUT is QT:
                    dst = QT0s[s][:, :]
                elif OUT is KT:
                    dst = KT[t][s][:, :]
                else:
                    dst = OUT[t][:, 512 * s : 512 * s + 512]
                nc.vector.tensor_scalar_add(dst, ps[:], b_sb[:, t : t + 1])

            def emit_v_tile(i):
                ps = psA.tile([128, HL], F32, name=f"psv{i}", tag="psA")
                for c in range(NHC):
                    nc.tensor.matmul(
                        ps[:],
                        xv_sb[c][:, 128 * i : 128 * i + 128],
                        wv_sb[c][:],
                        start=(c == 0),
                        stop=(c == NHC - 1),
                    )
                nc.gpsimd.memset(V[i][:], 1.0)
                vv = V[i].rearrange("p (h e) -> p h e", e=128)
                nc.vector.tensor_add(
                    vv[:, :, 0:HD],
                    ps[:].rearrange("p (h d) -> p h d", d=HD),
                    bvb_sb[:].rearrange("p (h d) -> p h d", d=HD),
                )

            def emit_outproj(q):
                # output projection for one finished q-tile; reuses the psA
                # slots that the (by now finished) projections vacated.
                ot = dpool.tile([128, HIDDEN], F32, name=f"ot{q}", tag="ot", bufs=2)
                for half in range(2):
                    po = psA.tile([128, 512], F32, name=f"po{q}_{half}", tag="psA")
                    for tt in range(NT):
                        nc.tensor.matmul(
                            po[:],
                            CX[tt][:, 128 * q : 128 * q + 128],
                            wo_sb[tt][:, 512 * half : 512 * half + 512],
                            start=(tt == 0),
                            stop=(tt == NT - 1),
                        )
                    nc.vector.tensor_copy(ot[:, 512 * half : 512 * half + 512], po[:])
                nc.sync.dma_start(out=out_d[128 * q : 128 * q + 128, :], in_=ot[:])

            def emit_attention_chunk(t, s, jit_v=False, final=False, extra=None):
                sq = slice(512 * s, 512 * s + 512)
                if t == 0:
                    qt_lo, qt_hi = QT0s[s][0:64, :], QT0s[s][64:128, :]
                else:
                    qt_lo, qt_hi = QT[t][0:64, sq], QT[t][64:128, sq]
                ctx0 = psC.tile([128, 512], F32, name=f"c0_{t}{s}", tag="ctx0")
                ctx1 = psC.tile([128, 512], F32, name=f"c1_{t}{s}", tag="ctx1")
                for i in range(NI):
                    kc, ko = i // 4, i % 4
                    sk = slice(128 * ko, 128 * ko + 128)
                    st = psS.tile([128, 1024], F32, name=f"st{t}{s}{i}", tag="st")
                    nc.tensor.matmul(
                        st[:, 0:512],
                        KT[t][kc][0:64, sk],
                        qt_lo,
                        start=True,
                        stop=True,
                        tile_position=(0, 0),
                    )
                    nc.tensor.matmul(
                        st[:, 512:1024],
                        KT[t][kc][64:128, sk],
                        qt_hi,
                        start=True,
                        stop=True,
                        tile_position=(64, 0),
                    )
                    pt = dpool.tile([128, 1024], dt_mm, name=f"pt{t}{s}{i}", tag="pt", bufs=6)
                    nc.scalar.activation(
                        pt[:], st[:], mybir.ActivationFunctionType.Exp, scale=SCALE
                    )
                    if jit_v:
                        # produce V[i] just in time for its ctx matmul
                        emit_v_tile(i)
                    h0 = 2 * t
                    h1 = 2 * t + 1
                    nc.tensor.matmul(
                        ctx0[:],
                        V[i][:, 128 * h0 : 128 * h0 + 128],
                        pt[:, 0:512],
                        start=(i == 0),
                        stop=(i == NI - 1),
                    )
                    nc.tensor.matmul(
                        ctx1[:],
                        V[i][:, 128 * h1 : 128 * h1 + 128],
                        pt[:, 512:1024],
                        start=(i == 0),
                        stop=(i == NI - 1),
                    )
                if extra is not None:
                    # deferred work whose inputs were finalized during the
                    # first half of THIS chunk (e.g. the previous s-chunk's
                    # output projection): emitted after the whole i-loop so
                    # the in-order PE queue reaches it ~a chunk later.
                    extra()
                # copy out of psum promptly (frees the single ctx bank), then
                # normalize from SBUF: rows 64:128 hold the replicated
                # softmax denominator.
                cxu0 = dpool.tile([128, 512], F32, name=f"u0_{t}{s}", tag="cxu0", bufs=1)
                cxu1 = dpool.tile([128, 512], F32, name=f"u1_{t}{s}", tag="cxu1", bufs=1)
                nc.vector.tensor_copy(cxu0[:], ctx0[:])
                nc.vector.tensor_copy(cxu1[:], ctx1[:])
                rb0 = dpool.tile([64, 512], F32, name=f"rb0_{t}{s}", tag="rb0", bufs=1)
                rb1 = dpool.tile([64, 512], F32, name=f"rb1_{t}{s}", tag="rb1", bufs=1)
                if not final:
                    nc.vector.reciprocal(rb0[:], cxu0[64:128, :])
                    nc.vector.reciprocal(rb1[:], cxu1[64:128, :])
                    if t < NT - 1:
                        # normalize on the otherwise-idle gpsimd engine
                        nc.gpsimd.tensor_mul(CX[t][0:64, sq], cxu0[0:64, :], rb0[:])
                        nc.gpsimd.tensor_mul(CX[t][64:128, sq], cxu1[0:64, :], rb1[:])
                    else:
                        # t=3 feeds the interleaved output projection; keep
                        # the CX chain on the faster vector engine
                        nc.vector.tensor_mul(CX[t][0:64, sq], cxu0[0:64, :], rb0[:])
                        nc.vector.tensor_mul(CX[t][64:128, sq], cxu1[0:64, :], rb1[:])
                else:
                    # last chunk of the kernel: split the normalize in half
                    # and push each half's output projection as soon as its
                    # 256 columns of CX are final, shortening the tail.
                    for half in range(2):
                        cols = slice(256 * half, 256 * half + 256)
                        sqh = slice(512 * s + 256 * half, 512 * s + 256 * half + 256)
                        nc.vector.reciprocal(rb0[:, cols], cxu0[64:128, cols])
                        nc.vector.reciprocal(rb1[:, cols], cxu1[64:128, cols])
                        nc.vector.tensor_mul(CX[t][0:64, sqh], cxu0[0:64, cols], rb0[:, cols])
                        nc.vector.tensor_mul(CX[t][64:128, sqh], cxu1[0:64, cols], rb1[:, cols])
                        for q in range(4 * s + 2 * half, 4 * s + 2 * half + 2):
                            emit_outproj(q)

            PROJ_Q = (xqT_d, wq_sb, bq_sb, QT, "q")
            PROJ_K = (xkT_d, wk_sb, bk_sb, KT, "k")

            # ---- warm the ACT table (exp set loads take ~2.7us; do it
            # during the prologue DMA wait instead of on the first real exp)
            warm = dpool.tile([1, 16], F32, name="warm", tag="warm", bufs=1)
            nc.gpsimd.memset(warm[:], 0.0)
            nc.scalar.activation(warm[:], warm[:], mybir.ActivationFunctionType.Exp)

            # ---- prologue. Sync-queue DMAs in order of first use so the
            # first score matmuls (and hence the ACT pipeline) start as
            # early as possible; the V path (wv/bvb/xv) rides the second
            # hwdge queue (Activation), which is idle until the first exp.
            for c in range(NHC):
                nc.scalar.dma_start(out=wv_sb[c][:], in_=wvT_d[128 * c : 128 * c + 128, :])
            nc.scalar.dma_start(out=bvb_sb[:], in_=bvb_d[:])
            xv_sb = [
                spool.tile([128, SK], dt_mm, name=f"xv{c}", tag=f"xf{c}", bufs=1) for c in range(NHC)
            ]
            for c in range(NHC):
                nc.scalar.dma_start(out=xv_sb[c][:], in_=xvT_d[128 * c : 128 * c + 128, :])

            emit_w_dma(wq_sb, wqT_d, 0)
            nc.sync.dma_start(out=bq_sb[:], in_=bq_d[:])
            emit_proj(0, 0, PROJ_Q)
            emit_w_dma(wk_sb, wkT_d, 0)
            nc.sync.dma_start(out=bk_sb[:], in_=bk_d[:])
            for s in range(NS):
                emit_proj(0, s, PROJ_K)
            for s in range(1, NS):
                emit_proj(0, s, PROJ_Q)
            for t in range(NT):
                nc.sync.dma_start(out=wo_sb[t][:], in_=woT_d[128 * t : 128 * t + 128, :])
            emit_attention_chunk(0, 0, jit_v=True)
            for s in range(1, NS):
                emit_attention_chunk(0, s)

            # ---- t>=1: emit projections for t, then attention(t); the
            # scheduler overlaps them into attention's ACT-bound gaps.
            # During the last head-pair (t=3) there are no more projections,
            # so the output projection of each finished s-chunk is emitted
            # at the end of the NEXT chunk (a full chunk of slack, so the
            # in-order PE queue never waits on the DVE epilogue that
            # finalizes CX).
            for t in range(1, NT):
                emit_w_dma(wq_sb, wqT_d, t)
                emit_w_dma(wk_sb, wkT_d, t)
                for s in range(NS):
                    emit_proj(t, s, PROJ_Q)
                    emit_proj(t, s, PROJ_K)
                for s in range(NS):
                    if t == NT - 1 and s >= 1:
                        prev = s - 1

                        def op_batch(prev=prev):
                            for q in range(4 * prev, 4 * prev + 4):
                                emit_outproj(q)

                        extra = op_batch
                    else:
                        extra = None
                    emit_attention_chunk(
                        t, s, final=(t == NT - 1 and s == NS - 1), extra=extra
                    )

    return nc


def _get_nc():
    dt_mm = F32 if os.environ.get("MHA_FP32") == "1" else BF16
    key = str(dt_mm)
    if key not in _CACHED:
        _CACHED[key] = _build_nc(dt_mm)
    return _CACHED[key], dt_mm


def kernel(query, key, value, Wq, bq, Wk, bk, Wv, bv, Wo, bo):
    nc, dt_mm = _get_nc()
    np_mm = ml_dtypes.bfloat16 if dt_mm == BF16 else np.float32

    query = np.asarray(query, dtype=np.float32)
    key = np.asarray(key, dtype=np.float32)
    value = np.asarray(value, dtype=np.float32)
    Wq = np.asarray(Wq, dtype=np.float32)
    Wk = np.asarray(Wk, dtype=np.float32)
    Wv = np.asarray(Wv, dtype=np.float32)
    Wo = np.asarray(Wo, dtype=np.float32)
    bq = np.asarray(bq, dtype=np.float32)
    bk = np.asarray(bk, dtype=np.float32)
    bv = np.asarray(bv, dtype=np.float32)
    bo = np.asarray(bo, dtype=np.float32)

    in_maps = []
    for c in range(NCORES):
        b_idx, hg = c // 2, c % 2
        rows = slice(HL * hg, HL * hg + HL)
        in_maps.append(
            {
                "xqT": np.ascontiguousarray(query[b_idx].T).astype(np_mm),
                "xkT": np.ascontiguousarray(key[b_idx].T).astype(np_mm),
                "xvT": np.ascontiguousarray(value[b_idx].T).astype(np_mm),
                "wqT": np.ascontiguousarray(Wq[rows, :].T).astype(np_mm),
                "wkT": np.ascontiguousarray(Wk[rows, :].T).astype(np_mm),
                "wvT": np.ascontiguousarray(Wv[rows, :].T).astype(np_mm),
                "woT": np.ascontiguousarray(Wo[:, rows].T).astype(np_mm),
                "bq2": np.ascontiguousarray(bq[rows].reshape(4, 128).T),
                "bk2": np.ascontiguousarray(bk[rows].reshape(4, 128).T),
                "bvb": np.ascontiguousarray(np.broadcast_to(bv[rows], (128, HL))),
                "out": None,
            }
        )
    for m in in_maps:
        del m["out"]

    trace = os.environ.get("MHA_TRACE") == "1"
    res = run_bass_kernel_spmd(nc, in_maps, list(range(NCORES)), trace=trace)
    if trace:
        kernel.last_exec_time_ns = res.exec_time_ns
        kernel.last_results = res

    out = np.empty((B, SQ, HIDDEN), dtype=np.float32)
    for b_idx in range(B):
        out[b_idx] = res.results[2 * b_idx]["out"]
        out[b_idx] += res.results[2 * b_idx + 1]["out"]
    out += bo[None, None, :]
    return out



# revision 18
# speedup vs baseline: 1.2412x; 1.2412x over previous
"""Multi-head attention (B=4, S=2048, H=1024, 16 heads) on 8 trn2 NeuronCores.

Sharding: data-parallel over batch (4) x tensor-parallel over head-groups (2):
core c handles batch c//2, heads 8*(c%2) .. 8*(c%2)+8. Each core computes its
partial output projection; host sums the two head-group partials + bo.

Per-core device algorithm (all matmul inputs bf16, fp32 accumulation):
  inputs are pre-transposed/packed on host: xqp/xkp hold x^T slices laid out
  so each (t,s) projection needs ONE [128,4096] DMA; wqk packs the per-
  head-pair Q and K weight slices into one [128,2048] DMA per phase.
  QT[t] (128=2 heads' d, sq) = wqk-slices^T @ x-chunks (+bias)
  KT[t][s'] likewise (split per sk-chunk for fine-grained deps)
  V halves (pairs 01 / 23), per 128-sk tile: [values(64)|ones(64)] per head;
  the ones columns make the ctx matmul emit the softmax denominator
  replicated in psum rows 64:128 at zero PE cost (matmul time ~ N only).
  per head-pair t, sq-chunk s (512), sk-tile i (128):
     S^T = KT-slice^T @ QT-slice  (two heads row-packed, run concurrently)
     P^T = exp(S^T * 0.125)       (ACT, psum->sbuf, bf16 out)
     ctx (128, 512) += V-block^T @ P^T
  normalize: ctx psum -> sbuf copy, reciprocal of rows 64:128 (DVE),
     CX = ctx * recip (gpsimd, to keep DVE short)
  out (sq, 1024) = sum_t CX-chunks^T @ woT   -> DMA out (fp32)

Schedule: ACT (exp) is the hard floor (256 x ~1.15us). Phase t=0 pays the
V-pair-01 projection jit inside chunk (0,0) and interleaves K(0,s')
projections with DMA arrival; V pairs 23 and the t+1 Q/K projections hide
in the ACT-bound phases' PE slack; the output projection of s-chunk j is
emitted at the end of chunk (3, j+1) so the in-order PE stream reaches it
only after the DVE epilogue that finalizes CX has long finished.
"""
import os
import sys

sys.path.insert(0, "/opt/trn_rl_repo")

import numpy as np
import ml_dtypes

import concourse.bass as bass
import concourse.mybir as mybir
import concourse.tile as tile

# ---------------------------------------------------------------------------
# Walrus in this environment allows at most 1 sync wait per instruction (2 for
# EventSemaphore); Tile sometimes emits more (e.g. the exit drain). Hoist the
# extra waits onto EventSemaphore instructions inserted before the offender.
import json as _json


def _transform_bir_json(bir_bytes: bytes) -> bytes:
    bir = _json.loads(bir_bytes)
    changed = False
    ctr = 0
    for fn in bir.get("functions", []):
        for blk in fn.get("blocks", []):
            out = []
            for inst in blk.get("instructions", []):
                si = inst.get("sync_info") or {}
                waits = si.get("on_wait") or []
                cap = 2 if inst.get("opcode") == "EventSemaphore" else 1
                if len(waits) > cap:
                    changed = True
                    extra = waits[:-cap]
                    si["on_wait"] = waits[-cap:]
                    for i in range(0, len(extra), 2):
                        ctr += 1
                        out.append(
                            {
                                "debug": inst.get("debug"),
                                "engine": inst["engine"],
                                "ins": [],
                                "name": f"{inst['name']}_xw{ctr}",
                                "opcode": "EventSemaphore",
                                "outs": [],
                                "sync_info": {
                                    "on_update": [],
                                    "on_wait": extra[i : i + 2],
                                },
                            }
                        )
                out.append(inst)
            blk["instructions"] = out
    if not changed:
        return bir_bytes
    return _json.dumps(bir).encode()


def _apply_bir_patch():
    import concourse.bass_utils as bu
    import concourse.bass2jax as b2j

    if getattr(b2j, "_bir_waitfix_applied", False):
        return
    orig = bu.compile_bir_kernel

    def patched(bir_json, tmpdir, neff_name="file.neff"):
        return orig(_transform_bir_json(bir_json), tmpdir, neff_name)

    b2j.compile_bir_kernel = patched
    bu.compile_bir_kernel = patched
    b2j._bir_waitfix_applied = True


_apply_bir_patch()

from concourse.bass_utils import run_bass_kernel_spmd  # noqa: E402

# ---------------------------------------------------------------------------
HIDDEN = 1024
HEADS = 16
HD = 64  # head dim
B, SQ, SK = 4, 2048, 2048
NCORES = 8
HPC = 8  # heads per core (tensor-parallel over 2 head groups)
HL = HPC * HD  # local hidden slice = 512
SCALE = HD ** -0.5

F32 = mybir.dt.float32
BF16 = mybir.dt.bfloat16

_CACHED = {}


def _build_nc(dt_mm):
    nc = bass.Bass()
    xqp_d = nc.declare_dram_parameter("xqp", [128, 4 * 4096], dt_mm, isOutput=False)
    xkp_d = nc.declare_dram_parameter("xkp", [128, 4 * 4096], dt_mm, isOutput=False)
    xvT_d = nc.declare_dram_parameter("xvT", [HIDDEN, SK], dt_mm, isOutput=False)
    wqk_d = nc.declare_dram_parameter("wqk", [128, 4 * 2048], dt_mm, isOutput=False)
    wvp_d = nc.declare_dram_parameter("wvp", [128, 4096], dt_mm, isOutput=False)
    wop_d = nc.declare_dram_parameter("wop", [128, 4096], dt_mm, isOutput=False)
    bq_d = nc.declare_dram_parameter("bq2", [128, 4], F32, isOutput=False)
    bk_d = nc.declare_dram_parameter("bk2", [128, 4], F32, isOutput=False)
    bvb_d = nc.declare_dram_parameter("bvb", [128, HL], F32, isOutput=False)
    out_d = nc.declare_dram_parameter("out", [SQ, HIDDEN], F32, isOutput=True)

    NHC = HIDDEN // 128  # 8 hidden chunks
    NT = 4  # head-pair tiles (8 local heads -> 4 pairs of 64 rows)
    NS = 4  # sq chunks of 512
    NI = SK // 128  # 16 sk tiles

    with tile.TileContext(nc) as tc:
        from contextlib import ExitStack

        with ExitStack() as stack:
            wpool = stack.enter_context(tc.tile_pool(name="wpool", bufs=1))
            apool = stack.enter_context(tc.tile_pool(name="apool", bufs=1))

            # ---- persistent weights / biases (DMAs emitted at point of need)
            wqk_sb = [wpool.tile([128, 2048], dt_mm, name=f"wqk{t}", tag=f"wqk{t}") for t in range(NT)]
            wv_sb = wpool.tile([128, 4096], dt_mm, name="wvp", tag="wvp")
            wo_sb = wpool.tile([128, 4096], dt_mm, name="wop", tag="wop")
            bq_sb = wpool.tile([128, 4], F32)
            bk_sb = wpool.tile([128, 4], F32)
            bvb_sb = wpool.tile([128, HL], F32)

            # ---- persistent activations
            QT = [apool.tile([128, SQ], dt_mm, name=f"QT{t}", tag=f"QT{t}") for t in range(NT)]
            # t=0's Q tiles are split per s-chunk so QT(0,s) projections can
            # be scheduled during attention(0,s') without a same-tile
            # write-during-read hazard.
            QT0s = [apool.tile([128, 512], dt_mm, name=f"QT0s{s}", tag=f"QT0s{s}") for s in range(NS)]
            # KT split per sk-chunk: attention(t,s,i) depends only on the K
            # projection covering its own sk range.
            KT = [
                [
                    apool.tile([128, 512], dt_mm, name=f"KT{t}_{c}", tag=f"KT{t}_{c}")
                    for c in range(NS)
                ]
                for t in range(NT)
            ]
            # V in two halves (head pairs 01 / 23), [128, 512] each:
            # per head a 128-col block [0:64]=values, [64:128]=ones.
            VH = [
                [
                    apool.tile([128, 512], dt_mm, name=f"V{h}_{i}", tag=f"V{h}_{i}")
                    for i in range(NI)
                ]
                for h in range(2)
            ]
            CX = [apool.tile([128, SQ], dt_mm, name=f"CX{t}", tag=f"CX{t}") for t in range(NT)]

            inner = stack.enter_context(ExitStack())
            spool = inner.enter_context(tc.tile_pool(name="ldpool", bufs=2))
            dpool = inner.enter_context(tc.tile_pool(name="dpool", bufs=4))
            psA = inner.enter_context(tc.tile_pool(name="psA", bufs=2, space="PSUM"))
            psS = inner.enter_context(tc.tile_pool(name="psS", bufs=2, space="PSUM"))
            psC = inner.enter_context(tc.tile_pool(name="psC", bufs=1, space="PSUM"))

            def emit_x_dma(which, t, s):
                xp_d, off, b_sb, OUT, nm = which
                xch = spool.tile(
                    [128, 4096], dt_mm, name=f"x{nm}{t}{s}", tag="xch", bufs=3
                )
                nc.sync.dma_start(
                    out=xch[:], in_=xp_d[:, 4096 * s : 4096 * s + 4096]
                )
                return xch

            def emit_proj(t, s, which, xch=None):
                xp_d, off, b_sb, OUT, nm = which
                if xch is None:
                    xch = emit_x_dma(which, t, s)
                ps = psA.tile([128, 512], F32, name=f"ps{nm}{s}{t}", tag="psA")
                for c in range(NHC):
                    nc.tensor.matmul(
                        ps[:],
                        wqk_sb[t][:, off + 128 * c : off + 128 * c + 128],
                        xch[:, 512 * c : 512 * c + 512],
                        start=(c == 0),
                        stop=(c == NHC - 1),
                    )
                if t == 0 and OUT is QT:
                    dst = QT0s[s][:, :]
                elif OUT is KT:
                    dst = KT[t][s][:, :]
                else:
                    dst = OUT[t][:, 512 * s : 512 * s + 512]
                nc.vector.tensor_scalar_add(dst, ps[:], b_sb[:, t : t + 1])

            def emit_v_tile(h, i):
                # V half h (head pairs 2h,2h+1), sk-tile i: 8 accumulating
                # matmuls of N=256 into half a psA slot, then ones + bias.
                ps = psA.tile([128, 512], F32, name=f"psv{h}_{i}", tag="psA")
                for c in range(NHC):
                    nc.tensor.matmul(
                        ps[:, 0:256],
                        xv_sb[c][:, 128 * i : 128 * i + 128],
                        wv_sb[:, 512 * c + 256 * h : 512 * c + 256 * h + 256],
                        start=(c == 0),
                        stop=(c == NHC - 1),
                    )
                nc.gpsimd.memset(VH[h][i][:], 1.0)
                vv = VH[h][i].rearrange("p (h e) -> p h e", e=128)
                nc.vector.tensor_add(
                    vv[:, :, 0:HD],
                    ps[:, 0:256].rearrange("p (h d) -> p h d", d=HD),
                    bvb_sb[:, 256 * h : 256 * h + 256].rearrange("p (h d) -> p h d", d=HD),
                )

            def emit_outproj(q):
                # output projection for one finished q-tile; reuses the psA
                # slots that the (by now finished) projections vacated.
                ot = dpool.tile([128, HIDDEN], F32, name=f"ot{q}", tag="ot", bufs=2)
                for half in range(2):
                    po = psA.tile([128, 512], F32, name=f"po{q}_{half}", tag="psA")
                    for tt in range(NT):
                        nc.tensor.matmul(
                            po[:],
                            CX[tt][:, 128 * q : 128 * q + 128],
                            wo_sb[:, 1024 * tt + 512 * half : 1024 * tt + 512 * half + 512],
                            start=(tt == 0),
                            stop=(tt == NT - 1),
                        )
                    nc.vector.tensor_copy(ot[:, 512 * half : 512 * half + 512], po[:])
                nc.sync.dma_start(out=out_d[128 * q : 128 * q + 128, :], in_=ot[:])

            def emit_attention_chunk(t, s, hooks=None, final=False, extra=None):
                sq = slice(512 * s, 512 * s + 512)
                if t == 0:
                    qt_lo, qt_hi = QT0s[s][0:64, :], QT0s[s][64:128, :]
                else:
                    qt_lo, qt_hi = QT[t][0:64, sq], QT[t][64:128, sq]
                Vh = VH[t // 2]
                vb = 256 * (t % 2)  # pair block offset inside the half
                ctx0 = psC.tile([128, 512], F32, name=f"c0_{t}{s}", tag="ctx0")
                ctx1 = psC.tile([128, 512], F32, name=f"c1_{t}{s}", tag="ctx1")
                for i in range(NI):
                    if hooks and i in hooks:
                        hooks[i]()
                    kc, ko = i // 4, i % 4
                    sk = slice(128 * ko, 128 * ko + 128)
                    st = psS.tile([128, 1024], F32, name=f"st{t}{s}{i}", tag="st")
                    nc.tensor.matmul(
                        st[:, 0:512],
                        KT[t][kc][0:64, sk],
                        qt_lo,
                        start=True,
                        stop=True,
                        tile_position=(0, 0),
                    )
                    nc.tensor.matmul(
                        st[:, 512:1024],
                        KT[t][kc][64:128, sk],
                        qt_hi,
                        start=True,
                        stop=True,
                        tile_position=(64, 0),
                    )
                    pt = dpool.tile([128, 1024], dt_mm, name=f"pt{t}{s}{i}", tag="pt", bufs=7)
                    nc.scalar.activation(
                        pt[:], st[:], mybir.ActivationFunctionType.Exp, scale=SCALE
                    )
                    nc.tensor.matmul(
                        ctx0[:],
                        Vh[i][:, vb : vb + 128],
                        pt[:, 0:512],
                        start=(i == 0),
                        stop=(i == NI - 1),
                    )
                    nc.tensor.matmul(
                        ctx1[:],
                        Vh[i][:, vb + 128 : vb + 256],
                        pt[:, 512:1024],
                        start=(i == 0),
                        stop=(i == NI - 1),
                    )
                if extra is not None:
                    # deferred work whose inputs were finalized a chunk ago
                    # (the previous s-chunk's output projection): the
                    # in-order PE stream reaches it only after this chunk's
                    # 64 matmuls, so its cross-engine wait is long satisfied.
                    extra()
                # copy out of psum promptly (frees the single ctx bank), then
                # normalize from SBUF: rows 64:128 hold the replicated
                # softmax denominator.
                cxu0 = dpool.tile([128, 512], F32, name=f"u0_{t}{s}", tag="cxu0", bufs=1)
                cxu1 = dpool.tile([128, 512], F32, name=f"u1_{t}{s}", tag="cxu1", bufs=1)
                nc.vector.tensor_copy(cxu0[:], ctx0[:])
                nc.vector.tensor_copy(cxu1[:], ctx1[:])
                rb0 = dpool.tile([64, 512], F32, name=f"rb0_{t}{s}", tag="rb0", bufs=1)
                rb1 = dpool.tile([64, 512], F32, name=f"rb1_{t}{s}", tag="rb1", bufs=1)
                if t < NT - 1:
                    nc.vector.reciprocal(rb0[:], cxu0[64:128, :])
                    nc.vector.reciprocal(rb1[:], cxu1[64:128, :])
                    # normalize on the otherwise-idle gpsimd engine
                    nc.gpsimd.tensor_mul(CX[t][0:64, sq], cxu0[0:64, :], rb0[:])
                    nc.gpsimd.tensor_mul(CX[t][64:128, sq], cxu1[0:64, :], rb1[:])
                else:
                    # t=3 feeds the interleaved output projection: halve the
                    # reciprocals so the CX chain completes well before the
                    # PE stream reaches the next chunk's outproj batch; on
                    # the final chunk also push each half's outproj as soon
                    # as its 256 columns of CX are final.
                    for half in range(2):
                        cols = slice(256 * half, 256 * half + 256)
                        sqh = slice(512 * s + 256 * half, 512 * s + 256 * half + 256)
                        nc.vector.reciprocal(rb0[:, cols], cxu0[64:128, cols])
                        nc.vector.reciprocal(rb1[:, cols], cxu1[64:128, cols])
                        nc.gpsimd.tensor_mul(CX[t][0:64, sqh], cxu0[0:64, cols], rb0[:, cols])
                        nc.gpsimd.tensor_mul(CX[t][64:128, sqh], cxu1[0:64, cols], rb1[:, cols])
                        if final:
                            for q in range(4 * s + 2 * half, 4 * s + 2 * half + 2):
                                emit_outproj(q)

            PROJ_Q = (xqp_d, 0, bq_sb, QT, "q")
            PROJ_K = (xkp_d, 1024, bk_sb, KT, "k")

            # ---- warm the ACT exp table (~2.7us) during the prologue DMAs
            warm = dpool.tile([1, 16], F32, name="warm", tag="warm", bufs=1)
            nc.gpsimd.memset(warm[:], 0.0)
            nc.scalar.activation(warm[:], warm[:], mybir.ActivationFunctionType.Exp)

            # ---- prologue: one sync DMA queue, ordered by first use. The
            # first-exp path (wqk0, xq_s0, xk_s0) leads; the V path and the
            # remaining K/Q slices interleave behind it to track the
            # consumption order of chunk (0,0).
            nc.sync.dma_start(out=wqk_sb[0][:], in_=wqk_d[:, 0:2048])
            nc.sync.dma_start(out=bq_sb[:], in_=bq_d[:])
            nc.sync.dma_start(out=bk_sb[:], in_=bk_d[:])
            emit_proj(0, 0, PROJ_Q)
            emit_proj(0, 0, PROJ_K)
            nc.sync.dma_start(out=wv_sb[:], in_=wvp_d[:])
            nc.sync.dma_start(out=bvb_sb[:], in_=bvb_d[:])
            # sync-queue order tracks chunk (0,0)'s consumption: xk_s1 (for
            # scores i>=4) ahead of the bulky xv, then xk_s2/s3.
            xk_pre = {1: emit_x_dma(PROJ_K, 0, 1)}
            xv_sb = [
                spool.tile([128, SK], dt_mm, name=f"xv{c}", tag=f"xf{c}", bufs=1) for c in range(NHC)
            ]
            for c in range(NHC):
                nc.sync.dma_start(out=xv_sb[c][:], in_=xvT_d[128 * c : 128 * c + 128, :])
            xk_pre[2] = emit_x_dma(PROJ_K, 0, 2)
            xk_pre[3] = emit_x_dma(PROJ_K, 0, 3)

            # chunk (0,0): V pairs 01 jit per i-tile; K(0,s') projection
            # matmuls interleave where their (already enqueued) slices land.
            hooks00 = {}
            for i in range(NI):
                hooks00[i] = (lambda i=i: emit_v_tile(0, i))
            for sp, at in ((1, 1), (2, 5), (3, 9)):
                prev = hooks00[at]

                def combo(prev=prev, sp=sp):
                    emit_proj(0, sp, PROJ_K, xch=xk_pre[sp])
                    prev()

                hooks00[at] = combo
            emit_attention_chunk(0, 0, hooks=hooks00)

            # t=0 s>=1: Q(0,s) projected just ahead; V pairs 23 trickle in
            # (2 tiles per chunk) through every ACT-bound chunk up to (1,3).
            vb_sched = {(0, 1): [0, 1], (0, 2): [2, 3], (0, 3): [4, 5]}
            for s in range(1, NS):
                emit_proj(0, s, PROJ_Q)
                hooks = {}
                for j, vi in enumerate(vb_sched.get((0, s), [])):
                    hooks[4 + 8 * j] = (lambda vi=vi: emit_v_tile(1, vi))
                emit_attention_chunk(0, s, hooks=hooks)

            vb_sched = {
                (1, 0): [6, 7],
                (1, 1): [8, 9],
                (1, 2): [10, 11, 12],
                (1, 3): [13, 14, 15],
            }
            for t in range(1, NT):
                nc.sync.dma_start(
                    out=wqk_sb[t][:], in_=wqk_d[:, 2048 * t : 2048 * t + 2048]
                )
                if t == NT - 1:
                    nc.sync.dma_start(out=wo_sb[:], in_=wop_d[:])
                for s in range(NS):
                    emit_proj(t, s, PROJ_Q)
                    emit_proj(t, s, PROJ_K)
                for s in range(NS):
                    hooks = {}
                    for j, vi in enumerate(vb_sched.get((t, s), [])):
                        hooks[3 + 5 * j] = (lambda vi=vi: emit_v_tile(1, vi))
                    if t == NT - 1 and s >= 1:
                        prev = s - 1

                        def op_batch(prev=prev):
                            for q in range(4 * prev, 4 * prev + 4):
                                emit_outproj(q)

                        extra = op_batch
                    else:
                        extra = None
                    emit_attention_chunk(
                        t,
                        s,
                        hooks=hooks,
                        final=(t == NT - 1 and s == NS - 1),
                        extra=extra,
                    )

    return nc


def _get_nc():
    dt_mm = F32 if os.environ.get("MHA_FP32") == "1" else BF16
    key = str(dt_mm)
    if key not in _CACHED:
        _CACHED[key] = _build_nc(dt_mm)
    return _CACHED[key], dt_mm


def _pack_inputs(q_b, k_b, v_b, Wq, Wk, Wv, Wo, bq, bk, bv, rows, np_mm):
    """Build the packed per-core input map for one (batch, head-group)."""
    xqT = np.ascontiguousarray(q_b.T)  # [1024, 2048]
    xkT = np.ascontiguousarray(k_b.T)
    xvT = np.ascontiguousarray(v_b.T)
    wqT = Wq[rows, :].T  # [1024, 512]
    wkT = Wk[rows, :].T
    wvT = Wv[rows, :].T  # [1024, 512]
    woT = Wo[:, rows].T  # [512, 1024]

    def pack_x(xT):
        # [128, 4*4096]: slot (s, c) at [:, 4096*s + 512*c] = xT[128c:+128, 512s:+512]
        out = np.empty((128, 4 * 4096), dtype=np_mm)
        for s in range(4):
            for c in range(8):
                out[:, 4096 * s + 512 * c : 4096 * s + 512 * c + 512] = xT[
                    128 * c : 128 * c + 128, 512 * s : 512 * s + 512
                ]
        return out

    wqk = np.empty((128, 4 * 2048), dtype=np_mm)
    for t in range(4):
        for c in range(8):
            wqk[:, 2048 * t + 128 * c : 2048 * t + 128 * c + 128] = wqT[
                128 * c : 128 * c + 128, 128 * t : 128 * t + 128
            ]
            wqk[:, 2048 * t + 1024 + 128 * c : 2048 * t + 1024 + 128 * c + 128] = wkT[
                128 * c : 128 * c + 128, 128 * t : 128 * t + 128
            ]
    wvp = np.empty((128, 4096), dtype=np_mm)
    for c in range(8):
        wvp[:, 512 * c : 512 * c + 512] = wvT[128 * c : 128 * c + 128, :]
    wop = np.empty((128, 4096), dtype=np_mm)
    for t in range(4):
        wop[:, 1024 * t : 1024 * t + 1024] = woT[128 * t : 128 * t + 128, :]

    return {
        "xqp": pack_x(xqT),
        "xkp": pack_x(xkT),
        "xvT": xvT.astype(np_mm),
        "wqk": wqk,
        "wvp": wvp,
        "wop": wop,
        "bq2": np.ascontiguousarray(bq[rows].reshape(4, 128).T),
        "bk2": np.ascontiguousarray(bk[rows].reshape(4, 128).T),
        "bvb": np.ascontiguousarray(np.broadcast_to(bv[rows], (128, HL))),
    }


def kernel(query, key, value, Wq, bq, Wk, bk, Wv, bv, Wo, bo):
    nc, dt_mm = _get_nc()
    np_mm = ml_dtypes.bfloat16 if dt_mm == BF16 else np.float32

    query = np.asarray(query, dtype=np.float32)
    key = np.asarray(key, dtype=np.float32)
    value = np.asarray(value, dtype=np.float32)
    Wq = np.asarray(Wq, dtype=np.float32)
    Wk = np.asarray(Wk, dtype=np.float32)
    Wv = np.asarray(Wv, dtype=np.float32)
    Wo = np.asarray(Wo, dtype=np.float32)
    bq = np.asarray(bq, dtype=np.float32)
    bk = np.asarray(bk, dtype=np.float32)
    bv = np.asarray(bv, dtype=np.float32)
    bo = np.asarray(bo, dtype=np.float32)

    in_maps = []
    for c in range(NCORES):
        b_idx, hg = c // 2, c % 2
        rows = slice(HL * hg, HL * hg + HL)
        m = _pack_inputs(
            query[b_idx].astype(np_mm),
            key[b_idx].astype(np_mm),
            value[b_idx].astype(np_mm),
            Wq.astype(np_mm),
            Wk.astype(np_mm),
            Wv.astype(np_mm),
            Wo.astype(np_mm),
            bq,
            bk,
            bv,
            rows,
            np_mm,
        )
        in_maps.append(m)

    trace = os.environ.get("MHA_TRACE") == "1"
    res = run_bass_kernel_spmd(nc, in_maps, list(range(NCORES)), trace=trace)
    if trace:
        kernel.last_exec_time_ns = res.exec_time_ns
        kernel.last_results = res

    out = np.empty((B, SQ, HIDDEN), dtype=np.float32)
    for b_idx in range(B):
        out[b_idx] = res.results[2 * b_idx]["out"]
        out[b_idx] += res.results[2 * b_idx + 1]["out"]
    out += bo[None, None, :]
    return out


# revision 27
# speedup vs baseline: 1.2782x; 1.0298x over previous
"""Multi-head attention (B=4, S=2048, H=1024, 16 heads) on 8 trn2 NeuronCores.

Sharding: data-parallel over batch (4) x tensor-parallel over head-groups (2):
core c handles batch c//2, heads 8*(c%2) .. 8*(c%2)+8. Each core computes its
partial output projection; host sums the two head-group partials + bo.

Per-core device algorithm (all matmul inputs bf16, fp32 accumulation):
  inputs are pre-transposed/packed on host: xqp/xkp hold x^T slices laid out
  so each (t,s) projection needs ONE [128,4096] DMA; wqk packs the per-
  head-pair Q and K weight slices into one [128,2048] DMA per phase.
  QT[t] (128=2 heads' d, sq) = wqk-slices^T @ x-chunks (+bias)
  KT[t][s'] likewise (split per sk-chunk for fine-grained deps)
  V halves (pairs 01 / 23), per 128-sk tile: [values(64)|ones(64)] per head;
  the ones columns make the ctx matmul emit the softmax denominator
  replicated in psum rows 64:128 at zero PE cost (matmul time ~ N only).
  per head-pair t, sq-chunk s (512), sk-tile i (128):
     S^T = KT-slice^T @ QT-slice  (two heads row-packed, run concurrently)
     P^T = exp(S^T * 0.125)       (ACT, psum->sbuf, bf16 out)
     ctx (128, 512) += V-block^T @ P^T
  normalize: ctx psum -> sbuf copy, reciprocal of rows 64:128 (DVE),
     CX = ctx * recip (gpsimd, to keep DVE short)
  out (sq, 1024) = sum_t CX-chunks^T @ woT   -> DMA out (fp32)

Schedule: ACT (exp) is the hard floor (256 x ~1.15us). Phase t=0 pays the
V-pair-01 projection jit inside chunk (0,0) and interleaves K(0,s')
projections with DMA arrival; V pairs 23 and the t+1 Q/K projections hide
in the ACT-bound phases' PE slack; the output projection of s-chunk j is
emitted at the end of chunk (3, j+1) so the in-order PE stream reaches it
only after the DVE epilogue that finalizes CX has long finished.
"""
import os
import sys

sys.path.insert(0, "/opt/trn_rl_repo")

import numpy as np
import ml_dtypes

import concourse.bass as bass
import concourse.mybir as mybir
import concourse.tile as tile

# ---------------------------------------------------------------------------
# Walrus in this environment allows at most 1 sync wait per instruction (2 for
# EventSemaphore); Tile sometimes emits more (e.g. the exit drain). Hoist the
# extra waits onto EventSemaphore instructions inserted before the offender.
import json as _json


def _transform_bir_json(bir_bytes: bytes) -> bytes:
    bir = _json.loads(bir_bytes)
    changed = False
    ctr = 0
    for fn in bir.get("functions", []):
        for blk in fn.get("blocks", []):
            out = []
            for inst in blk.get("instructions", []):
                si = inst.get("sync_info") or {}
                waits = si.get("on_wait") or []
                cap = 2 if inst.get("opcode") == "EventSemaphore" else 1
                if len(waits) > cap:
                    changed = True
                    extra = waits[:-cap]
                    si["on_wait"] = waits[-cap:]
                    for i in range(0, len(extra), 2):
                        ctr += 1
                        out.append(
                            {
                                "debug": inst.get("debug"),
                                "engine": inst["engine"],
                                "ins": [],
                                "name": f"{inst['name']}_xw{ctr}",
                                "opcode": "EventSemaphore",
                                "outs": [],
                                "sync_info": {
                                    "on_update": [],
                                    "on_wait": extra[i : i + 2],
                                },
                            }
                        )
                out.append(inst)
            blk["instructions"] = out
    if not changed:
        return bir_bytes
    return _json.dumps(bir).encode()


def _apply_bir_patch():
    import concourse.bass_utils as bu
    import concourse.bass2jax as b2j

    if getattr(b2j, "_bir_waitfix_applied", False):
        return
    orig = bu.compile_bir_kernel

    def patched(bir_json, tmpdir, neff_name="file.neff"):
        return orig(_transform_bir_json(bir_json), tmpdir, neff_name)

    b2j.compile_bir_kernel = patched
    bu.compile_bir_kernel = patched
    b2j._bir_waitfix_applied = True


_apply_bir_patch()

from concourse.bass_utils import run_bass_kernel_spmd  # noqa: E402

# ---------------------------------------------------------------------------
HIDDEN = 1024
HEADS = 16
HD = 64  # head dim
B, SQ, SK = 4, 2048, 2048
NCORES = 8
HPC = 8  # heads per core (tensor-parallel over 2 head groups)
HL = HPC * HD  # local hidden slice = 512
SCALE = HD ** -0.5

F32 = mybir.dt.float32
BF16 = mybir.dt.bfloat16

_CACHED = {}


def _build_nc(dt_mm):
    nc = bass.Bass()
    xqp_d = nc.declare_dram_parameter("xqp", [128, 4 * 4096], dt_mm, isOutput=False)
    xkp_d = nc.declare_dram_parameter("xkp", [128, 4 * 4096], dt_mm, isOutput=False)
    xvT_d = nc.declare_dram_parameter("xvT", [HIDDEN, SK], dt_mm, isOutput=False)
    wqk_d = nc.declare_dram_parameter("wqk", [128, 4 * 2048], dt_mm, isOutput=False)
    wvp_d = nc.declare_dram_parameter("wvp", [128, 4096], dt_mm, isOutput=False)
    wop_d = nc.declare_dram_parameter("wop", [128, 4096], dt_mm, isOutput=False)
    bq_d = nc.declare_dram_parameter("bq2", [128, 4], F32, isOutput=False)
    bk_d = nc.declare_dram_parameter("bk2", [128, 4], F32, isOutput=False)
    bvb_d = nc.declare_dram_parameter("bvb", [128, HL], F32, isOutput=False)
    out_d = nc.declare_dram_parameter("out", [SQ, HIDDEN], F32, isOutput=True)

    NHC = HIDDEN // 128  # 8 hidden chunks
    NT = 4  # head-pair tiles (8 local heads -> 4 pairs of 64 rows)
    NS = 4  # sq chunks of 512
    NI = SK // 128  # 16 sk tiles

    with tile.TileContext(nc) as tc:
        from contextlib import ExitStack

        with ExitStack() as stack:
            wpool = stack.enter_context(tc.tile_pool(name="wpool", bufs=1))
            apool = stack.enter_context(tc.tile_pool(name="apool", bufs=1))

            # ---- persistent weights / biases (DMAs emitted at point of need)
            wqk_sb = [wpool.tile([128, 2048], dt_mm, name=f"wqk{t}", tag=f"wqk{t}") for t in range(NT)]
            wv_sb = wpool.tile([128, 4096], dt_mm, name="wvp", tag="wvp")
            wo_sb = wpool.tile([128, 4096], dt_mm, name="wop", tag="wop")
            bq_sb = wpool.tile([128, 4], F32)
            bk_sb = wpool.tile([128, 4], F32)
            bvb_sb = wpool.tile([128, HL], F32)

            # ---- persistent activations. All split per (t, s-chunk) so
            # every dependency is tile-granular: projections, epilogues and
            # deferred work can interleave into the attention chunks without
            # false write-during-read hazards.
            QT = [
                [
                    apool.tile([128, 512], dt_mm, name=f"QT{t}_{s}", tag=f"QT{t}_{s}")
                    for s in range(NS)
                ]
                for t in range(NT)
            ]
            KT = [
                [
                    apool.tile([128, 512], dt_mm, name=f"KT{t}_{c}", tag=f"KT{t}_{c}")
                    for c in range(NS)
                ]
                for t in range(NT)
            ]
            # V in two halves (head pairs 01 / 23), [128, 512] each:
            # per head a 128-col block [0:64]=values, [64:128]=ones.
            VH = [
                [
                    apool.tile([128, 512], dt_mm, name=f"V{h}_{i}", tag=f"V{h}_{i}")
                    for i in range(NI)
                ]
                for h in range(2)
            ]
            CX = [
                [
                    apool.tile([128, 512], dt_mm, name=f"CX{t}_{s}", tag=f"CX{t}_{s}")
                    for s in range(NS)
                ]
                for t in range(NT)
            ]

            inner = stack.enter_context(ExitStack())
            spool = inner.enter_context(tc.tile_pool(name="ldpool", bufs=2))
            dpool = inner.enter_context(tc.tile_pool(name="dpool", bufs=4))
            psA = inner.enter_context(tc.tile_pool(name="psA", bufs=2, space="PSUM"))
            psS = inner.enter_context(tc.tile_pool(name="psS", bufs=2, space="PSUM"))
            psC = inner.enter_context(tc.tile_pool(name="psC", bufs=1, space="PSUM"))

            def emit_x_dma(which, t, s):
                xp_d, off, b_sb, OUT, nm = which
                xch = spool.tile(
                    [128, 4096], dt_mm, name=f"x{nm}{t}{s}", tag="xch", bufs=3
                )
                nc.sync.dma_start(
                    out=xch[:], in_=xp_d[:, 4096 * s : 4096 * s + 4096]
                )
                return xch

            def emit_proj(t, s, which, xch=None):
                xp_d, off, b_sb, OUT, nm = which
                if xch is None:
                    xch = emit_x_dma(which, t, s)
                ps = psA.tile([128, 512], F32, name=f"ps{nm}{s}{t}", tag="psA")
                for c in range(NHC):
                    nc.tensor.matmul(
                        ps[:],
                        wqk_sb[t][:, off + 128 * c : off + 128 * c + 128],
                        xch[:, 512 * c : 512 * c + 512],
                        start=(c == 0),
                        stop=(c == NHC - 1),
                    )
                nc.vector.tensor_scalar_add(OUT[t][s][:, :], ps[:], b_sb[:, t : t + 1])

            def emit_v_tile(h, i):
                # V half h (head pairs 2h,2h+1), sk-tile i: 8 accumulating
                # matmuls of N=256 into half a psA slot, then ones + bias.
                # xv is split in sk-halves so the i<8 tiles only wait on the
                # first half of the (late-arriving) xv DMA stream.
                xvh = xv_sb[i // 8]
                io = 128 * (i % 8)
                ps = psA.tile([128, 512], F32, name=f"psv{h}_{i}", tag="psA")
                for c in range(NHC):
                    nc.tensor.matmul(
                        ps[:, 0:256],
                        xvh[c][:, io : io + 128],
                        wv_sb[:, 512 * c + 256 * h : 512 * c + 256 * h + 256],
                        start=(c == 0),
                        stop=(c == NHC - 1),
                    )
                nc.gpsimd.memset(VH[h][i][:], 1.0)
                vv = VH[h][i].rearrange("p (h e) -> p h e", e=128)
                nc.vector.tensor_add(
                    vv[:, :, 0:HD],
                    ps[:, 0:256].rearrange("p (h d) -> p h d", d=HD),
                    bvb_sb[:, 256 * h : 256 * h + 256].rearrange("p (h d) -> p h d", d=HD),
                )

            def emit_outproj(q):
                # output projection for one finished q-tile; reuses the psA
                # slots that the (by now finished) projections vacated.
                qs, qo = q // 4, 128 * (q % 4)
                ot = dpool.tile([128, HIDDEN], F32, name=f"ot{q}", tag="ot", bufs=2)
                for half in range(2):
                    po = psA.tile([128, 512], F32, name=f"po{q}_{half}", tag="psA")
                    for tt in range(NT):
                        nc.tensor.matmul(
                            po[:],
                            CX[tt][qs][:, qo : qo + 128],
                            wo_sb[:, 1024 * tt + 512 * half : 1024 * tt + 512 * half + 512],
                            start=(tt == 0),
                            stop=(tt == NT - 1),
                        )
                    nc.vector.tensor_copy(ot[:, 512 * half : 512 * half + 512], po[:])
                nc.sync.dma_start(out=out_d[128 * q : 128 * q + 128, :], in_=ot[:])

            def emit_attention_chunk(t, s, hooks=None, final=False):
                qt_lo, qt_hi = QT[t][s][0:64, :], QT[t][s][64:128, :]
                Vh = VH[t // 2]
                vb = 256 * (t % 2)  # pair block offset inside the half
                ctx0 = psC.tile([128, 512], F32, name=f"c0_{t}{s}", tag="ctx0")
                ctx1 = psC.tile([128, 512], F32, name=f"c1_{t}{s}", tag="ctx1")
                for i in range(NI):
                    kc, ko = i // 4, i % 4
                    sk = slice(128 * ko, 128 * ko + 128)
                    st = psS.tile([128, 1024], F32, name=f"st{t}{s}{i}", tag="st")
                    nc.tensor.matmul(
                        st[:, 0:512],
                        KT[t][kc][0:64, sk],
                        qt_lo,
                        start=True,
                        stop=True,
                        tile_position=(0, 0),
                    )
                    nc.tensor.matmul(
                        st[:, 512:1024],
                        KT[t][kc][64:128, sk],
                        qt_hi,
                        start=True,
                        stop=True,
                        tile_position=(64, 0),
                    )
                    pt = dpool.tile([128, 1024], dt_mm, name=f"pt{t}{s}{i}", tag="pt", bufs=7)
                    nc.scalar.activation(
                        pt[:], st[:], mybir.ActivationFunctionType.Exp, scale=SCALE
                    )
                    if hooks and i in hooks:
                        # deferred PE work (V tiles, next-phase projections,
                        # finished-chunk output projections) rides INSIDE the
                        # i-loop: the in-order PE stream reaches it at the
                        # ACT-paced rate, filling this chunk's PE slack
                        # without ever batching up at a phase boundary.
                        hooks[i]()
                    nc.tensor.matmul(
                        ctx0[:],
                        Vh[i][:, vb : vb + 128],
                        pt[:, 0:512],
                        start=(i == 0),
                        stop=(i == NI - 1),
                    )
                    nc.tensor.matmul(
                        ctx1[:],
                        Vh[i][:, vb + 128 : vb + 256],
                        pt[:, 512:1024],
                        start=(i == 0),
                        stop=(i == NI - 1),
                    )
                # copy out of psum promptly (frees the single ctx bank), then
                # normalize from SBUF: rows 64:128 hold the replicated
                # softmax denominator.
                cxu0 = dpool.tile([128, 512], F32, name=f"u0_{t}{s}", tag="cxu0", bufs=1)
                cxu1 = dpool.tile([128, 512], F32, name=f"u1_{t}{s}", tag="cxu1", bufs=1)
                nc.vector.tensor_copy(cxu0[:], ctx0[:])
                nc.vector.tensor_copy(cxu1[:], ctx1[:])
                rb0 = dpool.tile([64, 512], F32, name=f"rb0_{t}{s}", tag="rb0", bufs=1)
                rb1 = dpool.tile([64, 512], F32, name=f"rb1_{t}{s}", tag="rb1", bufs=1)
                if t < NT - 1:
                    nc.vector.reciprocal(rb0[:], cxu0[64:128, :])
                    nc.vector.reciprocal(rb1[:], cxu1[64:128, :])
                    # normalize on the otherwise-idle gpsimd engine
                    nc.gpsimd.tensor_mul(CX[t][s][0:64, :], cxu0[0:64, :], rb0[:])
                    nc.gpsimd.tensor_mul(CX[t][s][64:128, :], cxu1[0:64, :], rb1[:])
                else:
                    # t=3 feeds the interleaved output projection: halve the
                    # reciprocals so the CX chain completes well before the
                    # PE stream reaches the next chunk's outproj hooks; on
                    # the final chunk also push each half's outproj as soon
                    # as its 256 columns of CX are final.
                    for half in range(2):
                        cols = slice(256 * half, 256 * half + 256)
                        nc.vector.reciprocal(rb0[:, cols], cxu0[64:128, cols])
                        nc.vector.reciprocal(rb1[:, cols], cxu1[64:128, cols])
                        nc.gpsimd.tensor_mul(
                            CX[t][s][0:64, cols], cxu0[0:64, cols], rb0[:, cols]
                        )
                        nc.gpsimd.tensor_mul(
                            CX[t][s][64:128, cols], cxu1[0:64, cols], rb1[:, cols]
                        )
                        if final:
                            for q in range(4 * s + 2 * half, 4 * s + 2 * half + 2):
                                emit_outproj(q)

            PROJ_Q = (xqp_d, 0, bq_sb, QT, "q")
            PROJ_K = (xkp_d, 1024, bk_sb, KT, "k")

            # ---- warm the ACT exp table (~2.7us) during the prologue DMAs
            warm = dpool.tile([1, 16], F32, name="warm", tag="warm", bufs=1)
            nc.gpsimd.memset(warm[:], 0.0)
            nc.scalar.activation(warm[:], warm[:], mybir.ActivationFunctionType.Exp)

            # ---- prologue: one sync DMA queue, ordered by first use. The
            # first-exp path (wqk0, xq_s0, xk_s0) leads; the V path and the
            # remaining K/Q slices interleave behind it to track the
            # consumption order of chunk (0,0).
            def add_hook(hooks, i, fn):
                if i in hooks:
                    prev = hooks[i]

                    def combo(prev=prev, fn=fn):
                        prev()
                        fn()

                    hooks[i] = combo
                else:
                    hooks[i] = fn

            nc.sync.dma_start(out=wqk_sb[0][:], in_=wqk_d[:, 0:2048])
            nc.sync.dma_start(out=bq_sb[:], in_=bq_d[:])
            nc.sync.dma_start(out=bk_sb[:], in_=bk_d[:])
            emit_proj(0, 0, PROJ_Q)
            emit_proj(0, 0, PROJ_K)
            nc.sync.dma_start(out=wv_sb[:], in_=wvp_d[:])
            nc.sync.dma_start(out=bvb_sb[:], in_=bvb_d[:])
            # sync-queue order tracks chunk (0,0)'s consumption: xk_s1 (for
            # scores i>=4), then the first sk-half of xv (V tiles 0..7),
            # xk_s2, the second xv half, xk_s3.
            xk_pre = {1: emit_x_dma(PROJ_K, 0, 1)}
            xv_sb = [
                [
                    spool.tile([128, 1024], dt_mm, name=f"xv{h}_{c}", tag=f"xf{h}_{c}", bufs=1)
                    for c in range(NHC)
                ]
                for h in range(2)
            ]
            for c in range(NHC):
                nc.sync.dma_start(
                    out=xv_sb[0][c][:], in_=xvT_d[128 * c : 128 * c + 128, 0:1024]
                )
            xk_pre[2] = emit_x_dma(PROJ_K, 0, 2)
            for c in range(NHC):
                nc.sync.dma_start(
                    out=xv_sb[1][c][:], in_=xvT_d[128 * c : 128 * c + 128, 1024:2048]
                )
            xk_pre[3] = emit_x_dma(PROJ_K, 0, 3)

            # ---- chunk (0,0): V pairs 01 jit per i-tile; K(0,s')
            # projections land where their slices arrive; Q(0,1) at the end.
            hooks = {}
            for i in range(NI):
                add_hook(hooks, i, lambda i=i: emit_v_tile(0, i))
            add_hook(hooks, 1, lambda: emit_proj(0, 1, PROJ_K, xch=xk_pre[1]))
            add_hook(hooks, 5, lambda: emit_proj(0, 2, PROJ_K, xch=xk_pre[2]))
            add_hook(hooks, 9, lambda: emit_proj(0, 3, PROJ_K, xch=xk_pre[3]))
            add_hook(hooks, 12, lambda: emit_proj(0, 1, PROJ_Q))
            emit_attention_chunk(0, 0, hooks=hooks)
            # packed weights for the later phases (queue is idle from here;
            # phase-t projections start a full phase early via hooks)
            for t in range(1, NT):
                nc.sync.dma_start(
                    out=wqk_sb[t][:], in_=wqk_d[:, 2048 * t : 2048 * t + 2048]
                )
            nc.sync.dma_start(out=wo_sb[:], in_=wop_d[:])

            # ---- remaining chunks: every piece of deferred PE work (next
            # s-chunk Q, next-phase Q/K projections, V pairs 23, output
            # projections at t=3) is hooked into an i-slot of a chunk whose
            # phase has ACT slack. proj_sched[(t,s)] = list of (i, fn).
            def P(t, s, which):
                return lambda: emit_proj(t, s, which)

            def VB(i):
                return lambda: emit_v_tile(1, i)

            def OP(q):
                return lambda: emit_outproj(q)

            sched = {
                (0, 1): [(2, P(1, 0, PROJ_Q)), (5, P(1, 0, PROJ_K)), (8, P(0, 2, PROJ_Q))],
                (0, 2): [(2, P(1, 1, PROJ_Q)), (5, P(1, 1, PROJ_K)), (8, P(0, 3, PROJ_Q))],
                (0, 3): [
                    (2, P(1, 2, PROJ_Q)),
                    (5, P(1, 2, PROJ_K)),
                    (9, P(1, 3, PROJ_Q)),
                    (12, P(1, 3, PROJ_K)),
                ],
                (1, 0): [(3, VB(0)), (7, VB(1)), (11, VB(2))],
                (1, 1): [(2, P(2, 0, PROJ_Q)), (5, P(2, 0, PROJ_K)), (8, VB(3)), (11, VB(4)), (14, VB(5))],
                (1, 2): [(2, P(2, 1, PROJ_Q)), (5, P(2, 1, PROJ_K)), (8, VB(6)), (11, VB(7)), (14, VB(8))],
                (1, 3): [(2, P(2, 2, PROJ_Q)), (5, P(2, 2, PROJ_K)), (8, VB(9)), (11, VB(10)), (14, VB(11))],
                # NOTE: KT[t][kc] tiles are sk-chunks — EVERY chunk of phase
                # t reads all four from i=12 on, so K(t,3) must land before
                # chunk (t,0) reaches i=12 (Q(t,s) is per-chunk and can lag).
                (2, 0): [(0, VB(12)), (1, P(2, 3, PROJ_K)), (3, VB(13)), (5, VB(14)), (7, VB(15))],
                (2, 1): [(2, P(2, 3, PROJ_Q)), (9, P(3, 0, PROJ_Q)), (12, P(3, 0, PROJ_K))],
                (2, 2): [(2, P(3, 1, PROJ_Q)), (5, P(3, 1, PROJ_K)), (9, P(3, 2, PROJ_Q)), (12, P(3, 2, PROJ_K))],
                (2, 3): [(2, P(3, 3, PROJ_Q)), (5, P(3, 3, PROJ_K))],
                (3, 1): [(9, OP(0)), (11, OP(1)), (13, OP(2)), (15, OP(3))],
                (3, 2): [(9, OP(4)), (11, OP(5)), (13, OP(6)), (15, OP(7))],
                (3, 3): [(9, OP(8)), (11, OP(9)), (13, OP(10)), (15, OP(11))],
            }

            for t in range(NT):
                for s in range(NS):
                    if t == 0 and s == 0:
                        continue
                    hooks = {}
                    for i, fn in sched.get((t, s), []):
                        add_hook(hooks, i, fn)
                    emit_attention_chunk(
                        t, s, hooks=hooks, final=(t == NT - 1 and s == NS - 1)
                    )

    return nc


def _get_nc():
    dt_mm = F32 if os.environ.get("MHA_FP32") == "1" else BF16
    key = str(dt_mm)
    if key not in _CACHED:
        _CACHED[key] = _build_nc(dt_mm)
    return _CACHED[key], dt_mm


def _pack_inputs(q_b, k_b, v_b, Wq, Wk, Wv, Wo, bq, bk, bv, rows, np_mm):
    """Build the packed per-core input map for one (batch, head-group)."""
    xqT = np.ascontiguousarray(q_b.T)  # [1024, 2048]
    xkT = np.ascontiguousarray(k_b.T)
    xvT = np.ascontiguousarray(v_b.T)
    wqT = Wq[rows, :].T  # [1024, 512]
    wkT = Wk[rows, :].T
    wvT = Wv[rows, :].T  # [1024, 512]
    woT = Wo[:, rows].T  # [512, 1024]

    def pack_x(xT):
        # [128, 4*4096]: slot (s, c) at [:, 4096*s + 512*c] = xT[128c:+128, 512s:+512]
        out = np.empty((128, 4 * 4096), dtype=np_mm)
        for s in range(4):
            for c in range(8):
                out[:, 4096 * s + 512 * c : 4096 * s + 512 * c + 512] = xT[
                    128 * c : 128 * c + 128, 512 * s : 512 * s + 512
                ]
        return out

    wqk = np.empty((128, 4 * 2048), dtype=np_mm)
    for t in range(4):
        for c in range(8):
            wqk[:, 2048 * t + 128 * c : 2048 * t + 128 * c + 128] = wqT[
                128 * c : 128 * c + 128, 128 * t : 128 * t + 128
            ]
            wqk[:, 2048 * t + 1024 + 128 * c : 2048 * t + 1024 + 128 * c + 128] = wkT[
                128 * c : 128 * c + 128, 128 * t : 128 * t + 128
            ]
    wvp = np.empty((128, 4096), dtype=np_mm)
    for c in range(8):
        wvp[:, 512 * c : 512 * c + 512] = wvT[128 * c : 128 * c + 128, :]
    wop = np.empty((128, 4096), dtype=np_mm)
    for t in range(4):
        wop[:, 1024 * t : 1024 * t + 1024] = woT[128 * t : 128 * t + 128, :]

    return {
        "xqp": pack_x(xqT),
        "xkp": pack_x(xkT),
        "xvT": xvT.astype(np_mm),
        "wqk": wqk,
        "wvp": wvp,
        "wop": wop,
        "bq2": np.ascontiguousarray(bq[rows].reshape(4, 128).T),
        "bk2": np.ascontiguousarray(bk[rows].reshape(4, 128).T),
        "bvb": np.ascontiguousarray(np.broadcast_to(bv[rows], (128, HL))),
    }


def kernel(query, key, value, Wq, bq, Wk, bk, Wv, bv, Wo, bo):
    nc, dt_mm = _get_nc()
    np_mm = ml_dtypes.bfloat16 if dt_mm == BF16 else np.float32

    query = np.asarray(query, dtype=np.float32)
    key = np.asarray(key, dtype=np.float32)
    value = np.asarray(value, dtype=np.float32)
    Wq = np.asarray(Wq, dtype=np.float32)
    Wk = np.asarray(Wk, dtype=np.float32)
    Wv = np.asarray(Wv, dtype=np.float32)
    Wo = np.asarray(Wo, dtype=np.float32)
    bq = np.asarray(bq, dtype=np.float32)
    bk = np.asarray(bk, dtype=np.float32)
    bv = np.asarray(bv, dtype=np.float32)
    bo = np.asarray(bo, dtype=np.float32)

    in_maps = []
    for c in range(NCORES):
        b_idx, hg = c // 2, c % 2
        rows = slice(HL * hg, HL * hg + HL)
        m = _pack_inputs(
            query[b_idx].astype(np_mm),
            key[b_idx].astype(np_mm),
            value[b_idx].astype(np_mm),
            Wq.astype(np_mm),
            Wk.astype(np_mm),
            Wv.astype(np_mm),
            Wo.astype(np_mm),
            bq,
            bk,
            bv,
            rows,
            np_mm,
        )
        in_maps.append(m)

    trace = os.environ.get("MHA_TRACE") == "1"
    res = run_bass_kernel_spmd(nc, in_maps, list(range(NCORES)), trace=trace)
    if trace:
        kernel.last_exec_time_ns = res.exec_time_ns
        kernel.last_results = res

    out = np.empty((B, SQ, HIDDEN), dtype=np.float32)
    for b_idx in range(B):
        out[b_idx] = res.results[2 * b_idx]["out"]
        out[b_idx] += res.results[2 * b_idx + 1]["out"]
    out += bo[None, None, :]
    return out
